# revision 46
# baseline (speedup 1.0000x reference)
"""Trainium2 Bass kernel for nn_CenterRegressor (4-layer GraphSAGE, mean-agg).

Self-contained: takes FULL inputs, shards across 8 NeuronCores internally,
returns the FULL [50000, 3] float32 output.

v2 design (per core, nodes sharded 8 ways, N padded 50000->50176, npc=6272):
  - feat-major activations: hT [128(f%128), 2(f//128), npc] bf16 in SBUF.
    Dense matmuls are weight-stationary: lhsT = 128x128 weight chunk, moving
    operand = hT node-blocks of 512 columns -> few large matmuls, warm PE.
  - per layer: h -> fp8 node-major bounce (DMA-transpose + ScalarE cast) ->
    AllGather per src-half into h_rep HBM replica; edge-gather of 256B fp8
    rows (4 SWDGE queues) exactly as v1; segment-mean via fp8 one-hot
    [slots x dst] matmuls accumulating in PSUM; inv-degree folded into the
    PSUM->SBUF copy; agg transposed to feat-major via 2 PE transposes/tile.
  - LayerNorm+L2 stats via all-ones [128x128] matmuls over outT and outT^2
    (partition reduction -> [128, bw] tiles with the per-node stat broadcast
    across partitions for free); row math on VectorE; LN gamma/beta + SiLU
    fused into one ScalarE activation (per-partition scale/bias).
  - AllGather split per half (nodes [0,3072) and [3072,6272)) so next layer's
    half-0 gathers overlap the tail of the current layer.
  - output head feat-major -> out [3, npc] f32, transposed on host.
"""
import os
import sys
import types
import contextlib

import numpy as np

sys.path.insert(0, "/opt/trn_rl_repo")

import ml_dtypes  # noqa: E402
import concourse.bacc as bacc  # noqa: E402
import concourse.bass as bass  # noqa: E402
import concourse.mybir as mybir  # noqa: E402
import concourse.tile as tile  # noqa: E402
from concourse.bass_utils import run_bass_kernel_spmd  # noqa: E402
from concourse.library_config import mlp  # noqa: E402

BF16 = ml_dtypes.bfloat16
FP8 = ml_dtypes.float8_e4m3fn
AF = mybir.ActivationFunctionType
DT = mybir.dt

LN_EPS = 1e-5
L2_EPS = 1e-12
MAX_CHUNKS_PER_CALL = 8  # <=1024 idxs per dma_gather call (2048 hangs NRT)

W = 8
NPC = 6272
NPAD = W * NPC
T = 49
H = 256
KC = 2
L = 4
IN_DIM = 16
HB = 3072                 # half boundary (within-shard node index)
NH = (HB, NPC - HB)       # rows per half per core (3072, 3200)
REG0 = W * NH[0]          # h_rep rows in region 0 (24576)
BLOCKS = [(i * 512, 512) for i in range(12)] + [(6144, 128)]
H0_LAST_BLOCK = 5         # blocks 0..5 cover nodes [0, 3072)


def _enable_axon_profile():
    if "antenv.axon_hooks" not in sys.modules:
        mod = types.ModuleType("antenv.axon_hooks")
        holder = [None]
        mod.set_axon_ntff_profile_hook = lambda h: holder.__setitem__(0, h)
        mod.get_axon_ntff_profile_hook = lambda: holder[0]
        sys.modules["antenv.axon_hooks"] = mod
        import antenv
        antenv.axon_hooks = mod
        try:
            from trn_agent_boot.trn_boot import _ntff_profile_via_ctypes
            mod.set_axon_ntff_profile_hook(
                _ntff_profile_via_ctypes("/opt/axon/libaxon_pjrt.so"))
        except Exception:
            pass
    import concourse.bass_utils as bu
    bu.upload_artifacts = lambda tmpdir: f"file://{tmpdir}"


def _idx_layout(flat):
    """int16 [S] -> [128, S/16] wrapped in 16 partitions, replicated x8."""
    s = flat.shape[0]
    assert s % 16 == 0
    return np.tile(flat.reshape(s // 16, 16).T, (8, 1)).astype(np.int16)


def preprocess(edge_index):
    """Build the uniform SPMD schedule + per-core gather/one-hot data.

    Slot order: all half-0 chunks (by tile), then all half-1 chunks.
    idx values are rows into the per-half h_rep region:
      region h row = w*NH[h] + (n_local - h*HB).
    """
    src = np.asarray(edge_index[0], dtype=np.int64)
    dst = np.asarray(edge_index[1], dtype=np.int64)

    deg = np.bincount(dst, minlength=NPAD).astype(np.float64)
    inv_deg = (1.0 / np.maximum(deg, 1.0)).astype(np.float32)

    core_of = dst // NPC
    counts = np.zeros((W, T, 2), dtype=np.int64)
    per_cth = {}
    for c in range(W):
        m = core_of == c
        s_c = src[m]
        dl = dst[m] - c * NPC
        t_c = dl // 128
        p_c = dl % 128
        w_c = s_c // NPC
        nl_c = s_c % NPC
        hf_c = (nl_c >= HB).astype(np.int64)
        row_c = w_c * (np.where(hf_c == 0, NH[0], NH[1])) + (nl_c - hf_c * HB)
        # sort by (tile, half), then by ascending source row within each
        # group so each gather call sweeps HBM quasi-monotonically
        key = t_c * 2 + hf_c
        order = np.lexsort((row_c, key))
        row_c, p_c, key = row_c[order], p_c[order], key[order]
        bounds = np.searchsorted(key, np.arange(2 * T + 1))
        for t in range(T):
            for hf in range(2):
                lo, hi = bounds[t * 2 + hf], bounds[t * 2 + hf + 1]
                per_cth[(c, t, hf)] = (row_c[lo:hi], p_c[lo:hi])
                counts[c, t, hf] = hi - lo

    K = np.ceil(counts / 128.0).astype(np.int64).max(axis=0)
    K = np.maximum(K, 1)  # >=1 chunk per (tile, half) so PSUM chains start

    nlo = int(K[:, 0].sum())
    nhi = int(K[:, 1].sum())
    total_chunks = nlo + nhi
    total_slots = total_chunks * 128
    cth_off = np.zeros((2, T + 1), dtype=np.int64)
    cth_off[0, 1:] = np.cumsum(K[:, 0])
    cth_off[1, 1:] = nlo + np.cumsum(K[:, 1])
    cth_off[1, 0] = nlo

    calls = []  # (chunk0, n_chunks, half)
    for hf in range(2):
        lo, hi = (0, nlo) if hf == 0 else (nlo, nlo + nhi)
        c = lo
        while c < hi:
            n = min(MAX_CHUNKS_PER_CALL, hi - c)
            calls.append((c, n, hf))
            c += n

    sched = dict(K=K, cth_off=cth_off, total_chunks=total_chunks,
                 total_slots=total_slots, calls=calls)

    percore = []
    for c in range(W):
        idx_flat = np.zeros(total_slots, dtype=np.int16)
        oh = np.zeros((128, total_chunks * 128), dtype=FP8)
        for t in range(T):
            for hf in range(2):
                rows, p = per_cth[(c, t, hf)]
                off = int(cth_off[hf, t]) * 128
                n = rows.shape[0]
                idx_flat[off:off + n] = rows.astype(np.int16)
                sl = np.arange(n) + off
                oh[sl % 128, (sl // 128) * 128 + p] = 1.0
        inv_sb = inv_deg[c * NPC:(c + 1) * NPC].reshape(T, 128).T.copy()
        percore.append(dict(idx=_idx_layout(idx_flat), oh=oh, inv=inv_sb))
    return sched, percore


def build_program(sched):
    K, cth_off, calls = sched["K"], sched["cth_off"], sched["calls"]
    total_chunks, total_slots = sched["total_chunks"], sched["total_slots"]
    ohmax = int((K[:, 0] + K[:, 1]).max())

    nc = bacc.Bacc("TRN2", debug=True, num_swdge_queues=4)
    f32, bf16, fp8, i16 = DT.float32, DT.bfloat16, DT.float8e4, DT.int16

    # ---- external IO ----
    xT_in = nc.dram_tensor("xT", [IN_DIM, NPC], bf16, kind="ExternalInput")
    idx_in = nc.dram_tensor("idx", [128, total_slots // 16], i16, kind="ExternalInput")
    oh_in = nc.dram_tensor("oh", [128, total_chunks * 128], fp8, kind="ExternalInput")
    inv_in = nc.dram_tensor("inv", [128, T], f32, kind="ExternalInput")
    wp_in = nc.dram_tensor("wp", [IN_DIM, KC, 128], bf16, kind="ExternalInput")
    wl_in = nc.dram_tensor("wl", [128, L * 4, 128], bf16, kind="ExternalInput")
    wr_in = nc.dram_tensor("wr", [128, L * 4, 128], bf16, kind="ExternalInput")
    w1_in = nc.dram_tensor("w1", [128, 4, 128], bf16, kind="ExternalInput")
    w2_in = nc.dram_tensor("w2", [128, KC, 3], bf16, kind="ExternalInput")
    # bias row: [bp(256) | bl(L*256) | b1(256) | b2(3)]
    NBIAS = (L + 2) * H + 3
    brow_in = nc.dram_tensor("brow", [1, NBIAS], bf16, kind="ExternalInput")
    # ln gamma/beta columns: g at col l*2+o, b at 2L + l*2+o
    gb_in = nc.dram_tensor("gb", [128, 4 * L], f32, kind="ExternalInput")
    onesrow_in = nc.dram_tensor("onesrow", [1, 512], bf16, kind="ExternalInput")
    allones_in = nc.dram_tensor("allones", [128, 128], bf16, kind="ExternalInput")
    ident_in = nc.dram_tensor("ident", [128, 128], bf16, kind="ExternalInput")
    out_ext = nc.dram_tensor("out", [3, NPC], f32, kind="ExternalOutput")

    # ---- internal DRAM ----
    bounce = nc.dram_tensor("bounce", [NPC, H], fp8)
    # double-buffered replica (parity = layer % 2) so the AllGather for layer
    # l+1 never write-after-read stalls on layer l's still-draining gathers
    h_rep = nc.dram_tensor("h_rep", [2 * NPAD, H], fp8,
                           addr_space="Shared" if W > 1 else "Local")

    with contextlib.ExitStack() as ctx:
        tc = ctx.enter_context(tile.TileContext(nc))
        const = ctx.enter_context(tc.tile_pool(name="const", bufs=1))
        gpool = ctx.enter_context(tc.tile_pool(name="gath", bufs=18))
        ohpool = ctx.enter_context(tc.tile_pool(name="ohp", bufs=4))
        aggp = ctx.enter_context(tc.tile_pool(name="aggp", bufs=3))
        atp = ctx.enter_context(tc.tile_pool(name="atp", bufs=2))
        otp = ctx.enter_context(tc.tile_pool(name="otp", bufs=2))
        sqp = ctx.enter_context(tc.tile_pool(name="sqp", bufs=2))
        zp = ctx.enter_context(tc.tile_pool(name="zp", bufs=2))
        vw = ctx.enter_context(tc.tile_pool(name="vw", bufs=2))
        trp = ctx.enter_context(tc.tile_pool(name="trp", bufs=2))
        nmp = ctx.enter_context(tc.tile_pool(name="nmp", bufs=2))
        obp = ctx.enter_context(tc.tile_pool(name="obp", bufs=2))
        po = ctx.enter_context(tc.tile_pool(name="po", bufs=2, space="PSUM"))
        pstat = ctx.enter_context(tc.tile_pool(name="pstat", bufs=1, space="PSUM"))
        pa = ctx.enter_context(tc.tile_pool(name="pa", bufs=1, space="PSUM"))
        pt = ctx.enter_context(tc.tile_pool(name="pt", bufs=2, space="PSUM"))

        nc.gpsimd.load_library(mlp)

        def load_const(name, dram, shape, dt):
            t = const.tile(shape, dt, tag=name)
            nc.sync.dma_start(t[:], dram[:])
            return t

        idx_sb = load_const("idx", idx_in, [128, total_slots // 16], i16)
        inv_sb = load_const("inv", inv_in, [128, T], f32)
        xT_sb = load_const("xT", xT_in, [IN_DIM, NPC], bf16)
        wp_sb = load_const("wp", wp_in, [IN_DIM, KC, 128], bf16)
        wl_sb = load_const("wl", wl_in, [128, L * 4, 128], bf16)
        wr_sb = load_const("wr", wr_in, [128, L * 4, 128], bf16)
        w1_sb = load_const("w1", w1_in, [128, 4, 128], bf16)
        w2_sb = load_const("w2", w2_in, [128, KC, 3], bf16)
        brow_sb = load_const("brow", brow_in, [1, NBIAS], bf16)
        gb_sb = load_const("gb", gb_in, [128, 4 * L], f32)
        ones_sb = load_const("onesrow", onesrow_in, [1, 512], bf16)
        allones_sb = load_const("allones", allones_in, [128, 128], bf16)
        ident_sb = load_const("ident", ident_in, [128, 128], bf16)

        epsln = const.tile([128, 1], f32, tag="epsln")
        nc.vector.memset(epsln[:], LN_EPS)

        hT = const.tile([128, KC, NPC], bf16, tag="hT")

        def brow_ap(which, o):
            # slices of the bias row as [1, 128] stationaries
            if which == "b2":
                return brow_sb[:, (L + 2) * H:(L + 2) * H + 3]
            if which == "bp":
                base = o * 128
            elif which == "b1":
                base = (L + 1) * H + o * 128
            else:  # "bl<l>"
                base = H + int(which[2:]) * H + o * 128
            return brow_sb[:, base:base + 128]

        # ---- gather plumbing (lazy per-call issue, memoized per layer) ----
        gt_bufs = {}
        st = {"par": 0}  # h_rep parity of the current layer

        def ensure_call(ci):
            if ci in gt_bufs:
                return gt_bufs[ci]
            (c0, nch, hf) = calls[ci]
            gt = gpool.tile([128, MAX_CHUNKS_PER_CALL, H], fp8, tag="gt")
            nidx = nch * 128
            slot_off = c0 * 128
            base = st["par"] * NPAD
            src_ap = (h_rep[base:base + REG0, :] if hf == 0
                      else h_rep[base + REG0:base + NPAD, :])
            nc.gpsimd.dma_gather(
                gt[:, 0:nch, :], src_ap,
                idx_sb[:, slot_off // 16:(slot_off + nidx) // 16],
                nidx, nidx, H,
                single_packet=True, queue_num=ci % 4)
            gt_bufs[ci] = gt
            return gt

        call_of_chunk = {}
        for ci, (c0, nch, hf) in enumerate(calls):
            for j in range(nch):
                call_of_chunk[c0 + j] = (ci, j)

        def agg_tile(t):
            """Gather + one-hot matmul + inv-deg scale -> agg [128,H] bf16."""
            chunks = (list(range(int(cth_off[0, t]), int(cth_off[0, t + 1]))) +
                      list(range(int(cth_off[1, t]), int(cth_off[1, t + 1]))))
            ktot = len(chunks)
            oh_t = ohpool.tile([128, ohmax * 128], fp8, tag="oh")
            lo0, lo1 = int(cth_off[0, t]), int(cth_off[0, t + 1])
            hi0, hi1 = int(cth_off[1, t]), int(cth_off[1, t + 1])
            nlo_t = lo1 - lo0
            # oh loads ride the ACT HWDGE ring; bounce/transpose use SP's
            nc.scalar.dma_start(oh_t[:, 0:nlo_t * 128], oh_in[:, lo0 * 128:lo1 * 128])
            nc.scalar.dma_start(oh_t[:, nlo_t * 128:ktot * 128],
                                oh_in[:, hi0 * 128:hi1 * 128])
            psum_a = pa.tile([128, H], f32, tag="psa")
            mms = []
            i = 0
            while i < ktot:
                ci, j = call_of_chunk[chunks[i]]
                if (i + 1 < ktot and j + 1 < MAX_CHUNKS_PER_CALL
                        and chunks[i + 1] == chunks[i] + 1
                        and call_of_chunk[chunks[i + 1]] == (ci, j + 1)):
                    mms.append((ci, j, i, True))
                    i += 2
                else:
                    mms.append((ci, j, i, False))
                    i += 1
            for n, (ci, j, i, pair) in enumerate(mms):
                gt = ensure_call(ci)
                st, sp = (n == 0), (n == len(mms) - 1)
                if pair:
                    oh_ap = oh_t[:, i * 128:(i + 2) * 128].rearrange(
                        "p (k d) -> p k d", k=2)
                    nc.tensor.matmul(
                        psum_a[:], oh_ap, gt[:, j:j + 2, :],
                        start=st, stop=sp,
                        perf_mode=mybir.MatmulPerfMode.DoubleRow)
                else:
                    nc.tensor.matmul(
                        psum_a[:], oh_t[:, i * 128:(i + 1) * 128], gt[:, j, :],
                        start=st, stop=sp)
            agg = aggp.tile([128, H], bf16, tag="agg")
            nc.scalar.activation(agg[:], psum_a[:], AF.Copy, scale=inv_sb[:, t:t + 1])
            return agg

        def bounce_dma(hf):
            """hT[:, :, half] -> node-major fp8 bounce rows (no collective).

            PE transposes instead of DMA-transpose: the xbar transpose DMA
            serializes against the gather stream (~25us stall per layer)."""
            lo = 0 if hf == 0 else HB
            n = NH[hf]
            nt = n // 128
            nm8 = nmp.tile([128, 25, H], fp8, tag="nm8")
            for tt in range(nt):
                for k in range(KC):
                    pst = pt.tile([128, 128], bf16, tag="pst")
                    nc.tensor.transpose(
                        pst[:], hT[:, k, lo + tt * 128:lo + (tt + 1) * 128],
                        ident_sb[:])
                    if (tt + k) % 2 == 0:
                        nc.scalar.activation(nm8[:, tt, k * 128:(k + 1) * 128],
                                             pst[:], AF.Copy)
                    else:
                        nc.vector.tensor_copy(nm8[:, tt, k * 128:(k + 1) * 128],
                                              pst[:])
            dst = bounce[lo:lo + n, :].rearrange("(t p) c -> p t c", p=128)
            nc.sync.dma_start(dst, nm8[:, 0:nt, :])

        def ag_half(hf, par):
            """AllGather one bounce half into the parity-`par` h_rep region."""
            lo = 0 if hf == 0 else HB
            n = NH[hf]
            reg_lo = par * NPAD + (0 if hf == 0 else REG0)
            if W > 1:
                out_v = h_rep[reg_lo:reg_lo + W * n, :].rearrange(
                    "(w n) h -> w n h", w=W)
                nc.gpsimd.collective_compute(
                    "AllGather", mybir.AluOpType.bypass,
                    replica_groups=[list(range(W))],
                    ins=[bounce[lo:lo + n, :].opt()],
                    outs=[out_v.opt()])
            else:
                nc.sync.dma_start(h_rep[reg_lo:reg_lo + n, :], bounce[lo:lo + n, :])

        def dense_block(l, b):
            """out = agg@Wl + bl + h@Wr for node block b, feat-major; then
            L2-normalize + LayerNorm + SiLU + residual into hT."""
            n0, bw = BLOCKS[b]
            tiles = list(range(n0 // 128, (n0 + bw) // 128))
            atb = atp.tile([128, KC, 512], bf16, tag="atb")
            for ti, t in enumerate(tiles):
                agg = agg_tile(t)
                for k in range(KC):
                    pst = pt.tile([128, 128], bf16, tag="pst")
                    nc.tensor.transpose(pst[:], agg[:, k * 128:(k + 1) * 128],
                                        ident_sb[:])
                    if k == 0:
                        nc.scalar.activation(
                            atb[:, k, ti * 128:(ti + 1) * 128], pst[:], AF.Copy)
                    else:
                        nc.vector.tensor_copy(
                            atb[:, k, ti * 128:(ti + 1) * 128], pst[:])
            otb = otp.tile([128, KC, 512], bf16, tag="otb")
            sqb = sqp.tile([128, KC, 512], bf16, tag="sqb")
            for o in range(KC):
                ps = po.tile([128, 512], f32, tag="ps")
                nc.tensor.matmul(ps[:, 0:bw], brow_ap(f"bl{l}", o),
                                 ones_sb[:, 0:bw], start=True, stop=False)
                for k in range(KC):
                    nc.tensor.matmul(ps[:, 0:bw], wr_sb[:, l * 4 + k * 2 + o, :],
                                     hT[:, k, n0:n0 + bw], start=False, stop=False)
                for k in range(KC):
                    nc.tensor.matmul(ps[:, 0:bw], wl_sb[:, l * 4 + k * 2 + o, :],
                                     atb[:, k, 0:bw], start=False, stop=(k == KC - 1))
                nc.scalar.activation(otb[:, o, 0:bw], ps[:, 0:bw], AF.Copy)
                nc.scalar.activation(sqb[:, o, 0:bw], ps[:, 0:bw], AF.Square)
            # stats: per-node sum / sum-of-squares broadcast to all partitions
            S_ps = pstat.tile([128, 512], f32, tag="S")
            SS_ps = pstat.tile([128, 512], f32, tag="SS")
            for o in range(KC):
                nc.tensor.matmul(S_ps[:, 0:bw], allones_sb[:], otb[:, o, 0:bw],
                                 start=(o == 0), stop=(o == KC - 1))
            for o in range(KC):
                nc.tensor.matmul(SS_ps[:, 0:bw], allones_sb[:], sqb[:, o, 0:bw],
                                 start=(o == 0), stop=(o == KC - 1))
            # row math (all tiles [128, bw], every partition identical).
            # L2-norm + LayerNorm fold exactly to T = 1/sqrt(var + eps*SS)
            #   = 1/sqrt(q*(1+eps*H) - mu^2), with q = SS/H, mu = S/H.
            mu = vw.tile([128, 512], f32, tag="mu")
            q = vw.tile([128, 512], f32, tag="q")
            t1 = vw.tile([128, 512], f32, tag="t1")
            u = vw.tile([128, 512], f32, tag="u")
            TtF = vw.tile([128, 512], f32, tag="TtF")
            Tt = vw.tile([128, 512], bf16, tag="Tt")
            MT = vw.tile([128, 512], bf16, tag="MT")
            nc.scalar.activation(mu[:, 0:bw], S_ps[:, 0:bw], AF.Copy, scale=1.0 / H)
            nc.scalar.activation(q[:, 0:bw], SS_ps[:, 0:bw], AF.Copy, scale=1.0 / H)
            nc.vector.tensor_mul(t1[:, 0:bw], mu[:, 0:bw], mu[:, 0:bw])
            nc.vector.scalar_tensor_tensor(
                t1[:, 0:bw], q[:, 0:bw], 1.0 + LN_EPS * H, t1[:, 0:bw],
                op0=mybir.AluOpType.mult, op1=mybir.AluOpType.subtract)
            nc.scalar.activation(u[:, 0:bw], t1[:, 0:bw], AF.Sqrt)
            nc.vector.reciprocal_approx_fast(TtF[:, 0:bw], u[:, 0:bw])
            nc.scalar.activation(Tt[:, 0:bw], TtF[:, 0:bw], AF.Copy)
            nc.vector.tensor_mul(MT[:, 0:bw], mu[:, 0:bw], TtF[:, 0:bw])
            for o in range(KC):
                z = zp.tile([128, 512], bf16, tag="z")
                nc.vector.tensor_mul(z[:, 0:bw], otb[:, o, 0:bw], Tt[:, 0:bw])
                nc.vector.tensor_sub(z[:, 0:bw], z[:, 0:bw], MT[:, 0:bw])
                hp = zp.tile([128, 512], bf16, tag="hp")
                nc.scalar.activation(hp[:, 0:bw], z[:, 0:bw], AF.Silu,
                                     scale=gb_sb[:, l * 2 + o:l * 2 + o + 1],
                                     bias=gb_sb[:, 2 * L + l * 2 + o:2 * L + l * 2 + o + 1])
                nc.vector.tensor_add(hT[:, o, n0:n0 + bw], hT[:, o, n0:n0 + bw],
                                     hp[:, 0:bw])

        def head_block(b):
            n0, bw = BLOCKS[b]
            sT = zp.tile([128, KC, 512], bf16, tag="sT")
            for o in range(KC):
                ps = po.tile([128, 512], f32, tag="ps")
                nc.tensor.matmul(ps[:, 0:bw], brow_ap("b1", o), ones_sb[:, 0:bw],
                                 start=True, stop=False)
                for k in range(KC):
                    nc.tensor.matmul(ps[:, 0:bw], w1_sb[:, k * 2 + o, :],
                                     hT[:, k, n0:n0 + bw],
                                     start=False, stop=(k == KC - 1))
                nc.scalar.activation(sT[:, o, 0:bw], ps[:, 0:bw], AF.Silu)
            ps3 = pstat.tile([3, 512], f32, tag="ps3")
            nc.tensor.matmul(ps3[:, 0:bw], brow_ap("b2", 0), ones_sb[:, 0:bw],
                             start=True, stop=False)
            for k in range(KC):
                nc.tensor.matmul(ps3[:, 0:bw], w2_sb[:, k, :], sT[:, k, 0:bw],
                                 start=False, stop=(k == KC - 1))
            ob = obp.tile([3, 512], f32, tag="ob")
            nc.vector.tensor_copy(ob[:, 0:bw], ps3[:, 0:bw])
            nc.sync.dma_start(out_ext[:, n0:n0 + bw], ob[:, 0:bw])

        # ---- input projection (feat-major) ----
        for b, (n0, bw) in enumerate(BLOCKS):
            for o in range(KC):
                ps = po.tile([128, 512], f32, tag="ps")
                nc.tensor.matmul(ps[:, 0:bw], brow_ap("bp", o), ones_sb[:, 0:bw],
                                 start=True, stop=False)
                nc.tensor.matmul(ps[:, 0:bw], wp_sb[:, o, :], xT_sb[:, n0:n0 + bw],
                                 start=False, stop=True)
                nc.scalar.activation(hT[:, o, n0:n0 + bw], ps[:, 0:bw], AF.Copy)
            if b == H0_LAST_BLOCK:
                bounce_dma(0)
        bounce_dma(1)
        ag_half(0, 0)
        ag_half(1, 0)

        # ---- layers ----
        for l in range(L):
            gt_bufs.clear()
            st["par"] = l % 2
            with nc.named_scope(f"layer{l}"):
                for b in range(len(BLOCKS)):
                    dense_block(l, b)
                    if l == L - 1:
                        head_block(b)
                    elif b == H0_LAST_BLOCK:
                        bounce_dma(0)
                if l < L - 1:
                    bounce_dma(1)
                    ag_half(0, (l + 1) % 2)
                    ag_half(1, (l + 1) % 2)

    nc.compile()
    return nc


def run_sharded(x, edge_index, Wp, bp, Wl, bl, Wr, ln_g, ln_b, W1, b1, W2, b2,
                W=8, trace=False, tmpdir=None):
    assert W == 8
    n_nodes, in_dim = x.shape
    assert (in_dim, Wl.shape[0], Wl.shape[1]) == (IN_DIM, L, H)

    x = np.asarray(x, dtype=np.float32)
    x_pad = np.zeros((NPAD, in_dim), dtype=np.float32)
    x_pad[:n_nodes] = x

    sched, percore = preprocess(np.asarray(edge_index))
    nc = build_program(sched)

    def b16(a):
        return np.asarray(a, dtype=np.float32).astype(BF16)

    # weight chunk layouts: [128(inf), l*4+k*2+o, 128(outf)]
    def chunk_lhsT(Wm):  # [L, 256, 256] -> [128, L*4, 128]
        Wm = np.asarray(Wm, np.float32).reshape(L, KC, 128, KC, 128)
        return np.ascontiguousarray(
            Wm.transpose(2, 0, 1, 3, 4).reshape(128, L * 4, 128)).astype(BF16)

    wl_h = chunk_lhsT(Wl)
    wr_h = chunk_lhsT(Wr)
    w1_h = np.ascontiguousarray(
        np.asarray(W1, np.float32).reshape(KC, 128, KC, 128)
        .transpose(1, 0, 2, 3).reshape(128, 4, 128)).astype(BF16)
    w2_h = np.ascontiguousarray(
        np.asarray(W2, np.float32).reshape(KC, 128, 3)
        .transpose(1, 0, 2)).astype(BF16)
    wp_h = np.ascontiguousarray(
        np.asarray(Wp, np.float32).reshape(IN_DIM, KC, 128)).astype(BF16)
    brow = np.concatenate([
        np.asarray(bp, np.float32).ravel(),
        np.asarray(bl, np.float32).ravel(),
        np.asarray(b1, np.float32).ravel(),
        np.asarray(b2, np.float32).ravel()]).reshape(1, -1).astype(BF16)
    # gb columns: g[l,o] at col l*2+o ; b[l,o] at 2L + l*2+o
    g_cols = np.asarray(ln_g, np.float32).reshape(L, KC, 128).transpose(2, 0, 1).reshape(128, L * KC)
    b_cols = np.asarray(ln_b, np.float32).reshape(L, KC, 128).transpose(2, 0, 1).reshape(128, L * KC)
    gb = np.ascontiguousarray(np.concatenate([g_cols, b_cols], axis=1)).astype(np.float32)
    onesrow = np.ones((1, 512), dtype=BF16)
    allones = np.ones((128, 128), dtype=BF16)
    ident = np.eye(128, dtype=np.float32).astype(BF16)

    in_maps = []
    for c in range(W):
        xT = np.ascontiguousarray(x_pad[c * NPC:(c + 1) * NPC].T).astype(BF16)
        in_maps.append(dict(
            xT=xT, idx=percore[c]["idx"], oh=percore[c]["oh"],
            inv=percore[c]["inv"], wp=wp_h, wl=wl_h, wr=wr_h, w1=w1_h, w2=w2_h,
            brow=brow, gb=gb, onesrow=onesrow, allones=allones, ident=ident))
    try:
        res = run_bass_kernel_spmd(nc, in_maps, core_ids=list(range(W)),
                                   trace=trace, tmpdir=tmpdir)
    except Exception:
        import time as _time
        _time.sleep(5)
        res = run_bass_kernel_spmd(nc, in_maps, core_ids=list(range(W)),
                                   trace=trace, tmpdir=tmpdir)
    out = np.concatenate([res.results[c]["out"].T for c in range(W)], axis=0)
    return out[:n_nodes].astype(np.float32), res


def kernel(**inputs):
    out, _ = run_sharded(
        inputs["x"], inputs["edge_index"], inputs["Wp"], inputs["bp"],
        inputs["Wl"], inputs["bl"], inputs["Wr"], inputs["ln_g"],
        inputs["ln_b"], inputs["W1"], inputs["b1"], inputs["W2"], inputs["b2"],
        trace=bool(os.environ.get("KERNEL_TRACE")))
    return out


if os.environ.get("KERNEL_TRACE"):
    _enable_axon_profile()
